# revision 9
# baseline (speedup 1.0000x reference)
"""MoE (8 experts, top-2, swiglu) Trainium2 kernel.

Strategy: expert-parallel across 8 NeuronCores — core e holds expert e's
weights and computes that expert's contribution for ALL 128 tokens densely;
the per-token routing coefficient (0 for unrouted tokens) is computed
on-device from the routing logits and applied to the expert output. The
host sums the 8 partial outputs (the "combine").

Memory optimization vs the fp32 version: weights, tokens and activations
are cast to bf16 on the host, halving the HBM traffic that dominates this
kernel (48MB -> 24MB per core). Matmuls accumulate in fp32 PSUM; end-to-end
relative error ~4e-3, inside the 2e-2 gate.

Per-core device program:
  MM1:  h[t, o-blk]  += hsT[k]^T @ w1T[k, o-blk]     (K=hidden, 8 chunks)
  swiglu: act = silu(h_up) * h_gate                  (bf16 out)
  PE-transpose act -> actT [i, t]
  MM2:  y[g] += actT[ki]^T @ w2T[ki, g]              (K=inter, 32 chunks,
                                                      g = 2 blocks of 512)
  y *= coef (per-token routing coefficient)

Schedule: one HWDGE queue (qSP) streams 2MB weight tiles continuously
(w1 b0..b3 first, then w2 ki-quarters interleaved with the rest of w1).
The PE's first instruction is gated on w1 block 2, letting the wire run
~3 tiles ahead; from then on the PE program is software-pipelined
(MM1 of block x | transpose of an earlier block | MM2-partial of a yet
earlier block) so the PE never stalls — stalls matter doubly here because
the tensor clock drops to the mid p-state (~1.2GHz) after an idle and
needs ~3us of continuous work to return to 2.4GHz. MM2 accumulates into
persistent PSUM banks; each accumulation group owns a full bank because
a group's `start` zero-fills the whole bank (two groups must never share
one). Small loads and output stores ride qActivation.
"""

import numpy as np

import concourse.bass as bass
import concourse.bacc as bacc
import concourse.mybir as mybir
from concourse.tile import TileContext
from concourse.bass_utils import run_bass_kernel_spmd
from concourse.masks import make_identity

TOKENS = 128
HIDDEN = 1024
INTER = 4096
NEXP = 8
NCORES = 8

KH = HIDDEN // 128          # 8   hidden contraction chunks
NB = INTER // 512           # 8   o-blocks of 512 (each has up + gate)
KI = INTER // 128           # 32  inter contraction chunks
KQ = 4                      # w2 DMA quarters along ki (8 chunks each)
HG = 2                      # output blocks of 512 (one PSUM bank each)
HGW = HIDDEN // HG          # 512

F32 = mybir.dt.float32
BF16 = mybir.dt.bfloat16


def build_bass():
    nc = bacc.Bacc(None, target_bir_lowering=False)

    hst = nc.declare_dram_parameter("hst", [128, KH, TOKENS], BF16, isOutput=False)
    w1s = nc.declare_dram_parameter("w1s", [NB, 128, 2, KH, 512], BF16, isOutput=False)
    w2s = nc.declare_dram_parameter(
        "w2s", [KQ, 128, KI // KQ, HG, HGW], BF16, isOutput=False
    )
    routing = nc.declare_dram_parameter("routing", [128, NEXP], F32, isOutput=False)
    rlogit = nc.declare_dram_parameter("rlogit", [128, 1], F32, isOutput=False)
    outp = nc.declare_dram_parameter("outp", [128, HIDDEN], F32, isOutput=True)

    with TileContext(nc) as tc:
        with (
            tc.tile_pool(name="singles", bufs=1) as singles,
            tc.tile_pool(name="small", bufs=1) as small,
            tc.tile_pool(name="w1pool", bufs=5) as w1pool,
            tc.tile_pool(name="w2pool", bufs=4) as w2pool,
            tc.tile_pool(name="sigpool", bufs=2) as sigpool,
            tc.tile_pool(name="actpool", bufs=2) as actpool,
            tc.tile_pool(name="outpool", bufs=2) as outpool,
            tc.tile_pool(name="psum_u", bufs=2, space="PSUM") as psum_u,
            tc.tile_pool(name="psum_g", bufs=2, space="PSUM") as psum_g,
            tc.tile_pool(name="psum_t", bufs=2, space="PSUM") as psum_t,
            tc.tile_pool(name="psum_y", bufs=1, space="PSUM") as psum_y,
        ):
            ident = singles.tile([128, 128], BF16)
            make_identity(nc, ident)

            # ---- qSP: token activations then the w1 stream ----
            hst_sb = singles.tile([128, KH, TOKENS], BF16)
            nc.sync.dma_start(out=hst_sb, in_=hst[:])

            w1tiles = [None] * NB
            w2tiles = [None] * KQ

            def load_w1(b):
                w1tiles[b] = w1pool.tile([128, 2, KH, 512], BF16, name="w1t")
                nc.sync.dma_start(out=w1tiles[b], in_=w1s[b])

            def load_w2(q):
                w2tiles[q] = w2pool.tile(
                    [128, KI // KQ, HG, HGW], BF16, name="w2t"
                )
                nc.scalar.dma_start(out=w2tiles[q], in_=w2s[q])

            for b in range(NB):
                load_w1(b)

            # ---- qAct: routing logits, then the w2 quarters in parallel
            # with qSP's w1 stream ----
            r_sb = small.tile([128, NEXP], F32)
            nc.scalar.dma_start(out=r_sb, in_=routing[:])
            rl_sb = small.tile([128, 1], F32)
            nc.scalar.dma_start(out=rl_sb, in_=rlogit[:])
            for q in range(KQ):
                load_w2(q)

            # ---- routing coefficient for this core's expert ----
            m1 = small.tile([128, 1], F32)
            nc.vector.reduce_max(out=m1, in_=r_sb, axis=mybir.AxisListType.X)
            # mask out (one) max element, take max again -> second max
            mask = small.tile([128, NEXP], F32)
            nc.vector.tensor_scalar(
                out=mask, in0=r_sb, scalar1=m1, scalar2=None,
                op0=mybir.AluOpType.is_ge,
            )
            negmask = small.tile([128, NEXP], F32)
            nc.vector.tensor_scalar(
                out=negmask, in0=mask, scalar1=-1.0e30, scalar2=None,
                op0=mybir.AluOpType.mult,
            )
            tmp = small.tile([128, NEXP], F32)
            nc.vector.tensor_tensor(
                out=tmp, in0=r_sb, in1=negmask, op=mybir.AluOpType.add
            )
            m2 = small.tile([128, 1], F32)
            nc.vector.reduce_max(out=m2, in_=tmp, axis=mybir.AxisListType.X)
            # selected iff this expert's logit >= second max
            sel = small.tile([128, 1], F32)
            nc.vector.tensor_tensor(
                out=sel, in0=rl_sb, in1=m2, op=mybir.AluOpType.is_ge
            )
            rlm = small.tile([128, 1], F32)
            nc.vector.tensor_tensor(
                out=rlm, in0=rl_sb, in1=m1, op=mybir.AluOpType.subtract
            )
            m2m = small.tile([128, 1], F32)
            nc.vector.tensor_tensor(
                out=m2m, in0=m2, in1=m1, op=mybir.AluOpType.subtract
            )
            num = small.tile([128, 1], F32)
            nc.scalar.activation(
                out=num, in_=rlm, func=mybir.ActivationFunctionType.Exp,
            )
            den = small.tile([128, 1], F32)
            nc.scalar.activation(
                out=den, in_=m2m, func=mybir.ActivationFunctionType.Exp,
            )
            nc.vector.tensor_scalar(
                out=den, in0=den, scalar1=1.0, scalar2=None,
                op0=mybir.AluOpType.add,
            )
            rden = small.tile([128, 1], F32)
            nc.vector.reciprocal(out=rden, in_=den)
            coef = small.tile([128, 1], F32)
            nc.vector.tensor_tensor(
                out=coef, in0=num, in1=sel, op=mybir.AluOpType.mult
            )
            nc.vector.tensor_tensor(
                out=coef, in0=coef, in1=rden, op=mybir.AluOpType.mult
            )

            actT = singles.tile([128, KI, TOKENS], BF16)
            py = psum_y.tile([128, HG, HGW], F32)

            acts = [None] * NB
            mm2_first = [True]

            def mm1(b):
                """MM1 block b: up+gate matmuls, swiglu on scalar/vector,
                leaving act[b] in SBUF (bf16)."""
                w1t = w1tiles[b]
                pu = psum_u.tile([128, 512], F32)
                for k in range(KH):
                    nc.tensor.matmul(
                        pu, lhsT=hst_sb[:, k, :], rhs=w1t[:, 0, k, :],
                        start=(k == 0), stop=(k == KH - 1),
                    )
                pg = psum_g.tile([128, 512], F32)
                for k in range(KH):
                    nc.tensor.matmul(
                        pg, lhsT=hst_sb[:, k, :], rhs=w1t[:, 1, k, :],
                        start=(k == 0), stop=(k == KH - 1),
                    )
                # silu(x) = x * sigmoid(x); then gate multiply, cast to bf16
                sig = sigpool.tile([128, 512], F32)
                nc.scalar.activation(
                    out=sig, in_=pu, func=mybir.ActivationFunctionType.Sigmoid
                )
                sil = sigpool.tile([128, 512], F32)
                nc.vector.tensor_tensor(
                    out=sil, in0=sig, in1=pu, op=mybir.AluOpType.mult
                )
                act = actpool.tile([128, 512], BF16)
                nc.vector.tensor_tensor(
                    out=act, in0=sil, in1=pg, op=mybir.AluOpType.mult
                )
                acts[b] = act

            def transp(b):
                """PE-transpose act block b into actT columns."""
                for jj in range(4):
                    pt = psum_t.tile([128, 128], BF16)
                    nc.tensor.transpose(
                        pt, acts[b][:, jj * 128:(jj + 1) * 128], ident
                    )
                    nc.vector.tensor_copy(out=actT[:, b * 4 + jj, :], in_=pt)

            def mm2_partial(bb, last=False):
                """Accumulate act block bb's 4 ki-chunks into both y banks;
                on the last block, scale by coef and store."""
                q, ko = divmod(bb, 2)
                for g in range(HG):
                    for kk in range(4):
                        nc.tensor.matmul(
                            py[:, g, :],
                            lhsT=actT[:, 4 * bb + kk, :],
                            rhs=w2tiles[q][:, 4 * ko + kk, g, :],
                            start=(mm2_first[0] and kk == 0),
                            stop=(last and kk == 3),
                        )
                    if last:
                        yt = outpool.tile([128, HGW], F32)
                        nc.vector.tensor_scalar(
                            out=yt, in0=py[:, g, :], scalar1=coef, scalar2=None,
                            op0=mybir.AluOpType.mult,
                        )
                        nc.scalar.dma_start(
                            out=outp[:, g * HGW:(g + 1) * HGW], in_=yt
                        )
                mm2_first[0] = False

            # ---- software-pipelined emission ----
            # A=mm1, B=transpose, C=mm2_partial; each B trails its A by one
            # unit (hides the scalar/vector swiglu latency), each C trails
            # its B. A2 leads so the PE start is gated on w1 block 2.
            mm1(2)
            mm1(0)
            transp(2)
            mm1(1)
            transp(0)
            mm1(3)
            transp(1)
            mm2_partial(0)
            mm1(4)
            transp(3)
            mm2_partial(1)
            mm1(5)
            transp(4)
            mm2_partial(2)
            mm1(6)
            transp(5)
            mm2_partial(3)
            mm1(7)
            transp(6)
            mm2_partial(4)
            transp(7)
            mm2_partial(5)
            mm2_partial(6)
            mm2_partial(7, last=True)

    nc.finalize()
    return nc


_NC = None


def _get_nc():
    global _NC
    if _NC is None:
        _NC = build_bass()
    return _NC


def prep_inputs(hidden_states, routing, w1, w2):
    """Host-side shard + relayout + bf16 cast. Returns in_maps for 8 cores."""
    import ml_dtypes

    bf16 = ml_dtypes.bfloat16
    hs = np.asarray(hidden_states, dtype=np.float32).astype(bf16)
    rt = np.ascontiguousarray(np.asarray(routing, dtype=np.float32))
    w1b = np.asarray(w1, dtype=np.float32).astype(bf16)
    w2b = np.asarray(w2, dtype=np.float32).astype(bf16)

    # hst[p, k, t] = hs[t, k*128+p]
    hst = np.ascontiguousarray(hs.T.reshape(KH, 128, TOKENS).transpose(1, 0, 2))
    # w1s[e, b, p, u, k, o_l] = w1[e, u*4096 + b*512 + o_l, k*128 + p]
    w1p = np.ascontiguousarray(
        w1b.reshape(NEXP, 2, NB, 512, KH, 128).transpose(0, 2, 5, 1, 4, 3)
    )
    # w2s[e, q, p, kl, g, h_l] = w2[e, g*512 + h_l, (q*8 + kl)*128 + p]
    w2p = np.ascontiguousarray(
        w2b.reshape(NEXP, HG, HGW, KQ, KI // KQ, 128).transpose(0, 3, 5, 4, 1, 2)
    )

    in_maps = []
    for c in range(NCORES):
        in_maps.append({
            "hst": hst,
            "w1s": w1p[c],
            "w2s": w2p[c],
            "routing": rt,
            "rlogit": np.ascontiguousarray(rt[:, c:c + 1]),
        })
    return in_maps


def kernel(hidden_states, routing, w1, w2):
    nc = _get_nc()
    in_maps = prep_inputs(hidden_states, routing, w1, w2)
    res = run_bass_kernel_spmd(nc, in_maps, list(range(NCORES)))
    out = np.zeros((TOKENS, HIDDEN), dtype=np.float32)
    for c in range(NCORES):
        out += res.results[c]["outp"]
    return out


# revision 10
# speedup vs baseline: 1.1366x; 1.1366x over previous
"""MoE (8 experts, top-2, swiglu) Trainium2 kernel.

Strategy: expert-parallel across 8 NeuronCores — core e holds expert e's
weights and computes that expert's contribution for ALL 128 tokens densely;
the per-token routing coefficient (0 for unrouted tokens) is computed
on-device from the routing logits and applied to the expert output. The
host sums the 8 partial outputs (the "combine").

Memory optimization vs the fp32 version: weights, tokens and activations
are cast to bf16 on the host, halving the HBM traffic that dominates this
kernel (48MB -> 24MB per core). Matmuls accumulate in fp32 PSUM; end-to-end
relative error ~4e-3, inside the 2e-2 gate.

Per-core device program:
  MM1:  h[t, o-blk]  += hsT[k]^T @ w1T[k, o-blk]     (K=hidden, 8 chunks)
  swiglu: act = silu(h_up) * h_gate                  (bf16 out)
  PE-transpose act -> actT [i, t]
  MM2:  y[g] += actT[ki]^T @ w2T[ki, g]              (K=inter, 32 chunks,
                                                      g = 2 blocks of 512)
  y *= coef (per-token routing coefficient)

Schedule: one HWDGE queue (qSP) streams 2MB weight tiles continuously
(w1 b0..b3 first, then w2 ki-quarters interleaved with the rest of w1).
The PE's first instruction is gated on w1 block 2, letting the wire run
~3 tiles ahead; from then on the PE program is software-pipelined
(MM1 of block x | transpose of an earlier block | MM2-partial of a yet
earlier block) so the PE never stalls — stalls matter doubly here because
the tensor clock drops to the mid p-state (~1.2GHz) after an idle and
needs ~3us of continuous work to return to 2.4GHz. MM2 accumulates into
persistent PSUM banks; each accumulation group owns a full bank because
a group's `start` zero-fills the whole bank (two groups must never share
one). Small loads and output stores ride qActivation.
"""

import numpy as np

import concourse.bass as bass
import concourse.bacc as bacc
import concourse.mybir as mybir
from concourse.tile import TileContext
from concourse.bass_utils import run_bass_kernel_spmd
from concourse.masks import make_identity

TOKENS = 128
HIDDEN = 1024
INTER = 4096
NEXP = 8
NCORES = 8

KH = HIDDEN // 128          # 8   hidden contraction chunks
NB = INTER // 512           # 8   o-blocks of 512 (each has up + gate)
KI = INTER // 128           # 32  inter contraction chunks
KQ = 4                      # w2 DMA quarters along ki (8 chunks each)
HG = 2                      # output blocks of 512 (one PSUM bank each)
HGW = HIDDEN // HG          # 512

F32 = mybir.dt.float32
BF16 = mybir.dt.bfloat16


def build_bass():
    nc = bacc.Bacc(None, target_bir_lowering=False)

    hst = nc.declare_dram_parameter("hst", [128, KH, TOKENS], BF16, isOutput=False)
    w1s = nc.declare_dram_parameter("w1s", [NB, 128, 2, KH, 512], BF16, isOutput=False)
    w2s = nc.declare_dram_parameter(
        "w2s", [KQ, 128, KI // KQ, HG, HGW], BF16, isOutput=False
    )
    routing = nc.declare_dram_parameter("routing", [128, NEXP], F32, isOutput=False)
    rlogit = nc.declare_dram_parameter("rlogit", [128, 1], F32, isOutput=False)
    outp = nc.declare_dram_parameter("outp", [128, HIDDEN], F32, isOutput=True)

    with TileContext(nc) as tc:
        with (
            tc.tile_pool(name="singles", bufs=1) as singles,
            tc.tile_pool(name="small", bufs=1) as small,
            tc.tile_pool(name="w1pool", bufs=5) as w1pool,
            tc.tile_pool(name="w2pool", bufs=4) as w2pool,
            tc.tile_pool(name="sigpool", bufs=2) as sigpool,
            tc.tile_pool(name="actpool", bufs=2) as actpool,
            tc.tile_pool(name="outpool", bufs=2) as outpool,
            tc.tile_pool(name="psum_u", bufs=2, space="PSUM") as psum_u,
            tc.tile_pool(name="psum_g", bufs=2, space="PSUM") as psum_g,
            tc.tile_pool(name="psum_t", bufs=2, space="PSUM") as psum_t,
            tc.tile_pool(name="psum_y", bufs=1, space="PSUM") as psum_y,
        ):
            ident = singles.tile([128, 128], BF16)
            make_identity(nc, ident)

            # ---- qAct: token activations + routing logits (tiny, early) ----
            hst_sb = singles.tile([128, KH, TOKENS], BF16)
            nc.scalar.dma_start(out=hst_sb, in_=hst[:])
            r_sb = small.tile([128, NEXP], F32)
            nc.scalar.dma_start(out=r_sb, in_=routing[:])
            rl_sb = small.tile([128, 1], F32)
            nc.scalar.dma_start(out=rl_sb, in_=rlogit[:])

            # ---- qSP: the single interleaved weight stream ----
            # order: b0 b1 b2 b3 q0 b4 q1 b5 b6 q2 b7 q3(4 sub-DMAs)
            # q3 is split so its MM2 consumers unblock progressively even if
            # the stream tail trickles out under heavy compute.
            w1tiles = [None] * NB
            w2tiles = [None] * KQ

            def load_w1(b):
                w1tiles[b] = w1pool.tile([128, 2, KH, 512], BF16, name="w1t")
                nc.sync.dma_start(out=w1tiles[b], in_=w1s[b])

            def load_w2(q, split=1):
                w2tiles[q] = w2pool.tile(
                    [128, KI // KQ, HG, HGW], BF16, name="w2t"
                )
                kl_per = (KI // KQ) // split
                for s in range(split):
                    nc.sync.dma_start(
                        out=w2tiles[q][:, s * kl_per:(s + 1) * kl_per],
                        in_=w2s[q, :, s * kl_per:(s + 1) * kl_per],
                    )

            load_w1(0)
            load_w1(1)
            load_w1(2)
            load_w1(3)
            load_w2(0)
            load_w1(4)
            load_w2(1)
            load_w1(5)
            load_w1(6)
            load_w2(2)
            load_w1(7)
            load_w2(3, split=4)

            # ---- routing coefficient for this core's expert ----
            m1 = small.tile([128, 1], F32)
            nc.vector.reduce_max(out=m1, in_=r_sb, axis=mybir.AxisListType.X)
            # mask out (one) max element, take max again -> second max
            mask = small.tile([128, NEXP], F32)
            nc.vector.tensor_scalar(
                out=mask, in0=r_sb, scalar1=m1, scalar2=None,
                op0=mybir.AluOpType.is_ge,
            )
            negmask = small.tile([128, NEXP], F32)
            nc.vector.tensor_scalar(
                out=negmask, in0=mask, scalar1=-1.0e30, scalar2=None,
                op0=mybir.AluOpType.mult,
            )
            tmp = small.tile([128, NEXP], F32)
            nc.vector.tensor_tensor(
                out=tmp, in0=r_sb, in1=negmask, op=mybir.AluOpType.add
            )
            m2 = small.tile([128, 1], F32)
            nc.vector.reduce_max(out=m2, in_=tmp, axis=mybir.AxisListType.X)
            # selected iff this expert's logit >= second max
            sel = small.tile([128, 1], F32)
            nc.vector.tensor_tensor(
                out=sel, in0=rl_sb, in1=m2, op=mybir.AluOpType.is_ge
            )
            rlm = small.tile([128, 1], F32)
            nc.vector.tensor_tensor(
                out=rlm, in0=rl_sb, in1=m1, op=mybir.AluOpType.subtract
            )
            m2m = small.tile([128, 1], F32)
            nc.vector.tensor_tensor(
                out=m2m, in0=m2, in1=m1, op=mybir.AluOpType.subtract
            )
            num = small.tile([128, 1], F32)
            nc.scalar.activation(
                out=num, in_=rlm, func=mybir.ActivationFunctionType.Exp,
            )
            den = small.tile([128, 1], F32)
            nc.scalar.activation(
                out=den, in_=m2m, func=mybir.ActivationFunctionType.Exp,
            )
            nc.vector.tensor_scalar(
                out=den, in0=den, scalar1=1.0, scalar2=None,
                op0=mybir.AluOpType.add,
            )
            rden = small.tile([128, 1], F32)
            nc.vector.reciprocal(out=rden, in_=den)
            coef = small.tile([128, 1], F32)
            nc.vector.tensor_tensor(
                out=coef, in0=num, in1=sel, op=mybir.AluOpType.mult
            )
            nc.vector.tensor_tensor(
                out=coef, in0=coef, in1=rden, op=mybir.AluOpType.mult
            )

            actT = singles.tile([128, KI, TOKENS], BF16)
            py = psum_y.tile([128, HG, HGW], F32)

            acts = [None] * NB
            mm2_first = [True]

            def mm1(b):
                """MM1 block b: up+gate matmuls, swiglu on scalar/vector,
                leaving act[b] in SBUF (bf16)."""
                w1t = w1tiles[b]
                pu = psum_u.tile([128, 512], F32)
                for k in range(KH):
                    nc.tensor.matmul(
                        pu, lhsT=hst_sb[:, k, :], rhs=w1t[:, 0, k, :],
                        start=(k == 0), stop=(k == KH - 1),
                    )
                pg = psum_g.tile([128, 512], F32)
                for k in range(KH):
                    nc.tensor.matmul(
                        pg, lhsT=hst_sb[:, k, :], rhs=w1t[:, 1, k, :],
                        start=(k == 0), stop=(k == KH - 1),
                    )
                # silu(x) = x * sigmoid(x); then gate multiply, cast to bf16
                sig = sigpool.tile([128, 512], F32)
                nc.scalar.activation(
                    out=sig, in_=pu, func=mybir.ActivationFunctionType.Sigmoid
                )
                sil = sigpool.tile([128, 512], F32)
                nc.vector.tensor_tensor(
                    out=sil, in0=sig, in1=pu, op=mybir.AluOpType.mult
                )
                act = actpool.tile([128, 512], BF16)
                nc.vector.tensor_tensor(
                    out=act, in0=sil, in1=pg, op=mybir.AluOpType.mult
                )
                acts[b] = act

            def transp(b):
                """PE-transpose act block b into actT columns."""
                for jj in range(4):
                    pt = psum_t.tile([128, 128], BF16)
                    nc.tensor.transpose(
                        pt, acts[b][:, jj * 128:(jj + 1) * 128], ident
                    )
                    nc.vector.tensor_copy(out=actT[:, b * 4 + jj, :], in_=pt)

            def mm2_partial(bb, last=False):
                """Accumulate act block bb's 4 ki-chunks into both y banks;
                on the last block, scale by coef and store."""
                q, ko = divmod(bb, 2)
                for g in range(HG):
                    for kk in range(4):
                        nc.tensor.matmul(
                            py[:, g, :],
                            lhsT=actT[:, 4 * bb + kk, :],
                            rhs=w2tiles[q][:, 4 * ko + kk, g, :],
                            start=(mm2_first[0] and kk == 0),
                            stop=(last and kk == 3),
                        )
                    if last:
                        yt = outpool.tile([128, HGW], F32)
                        nc.vector.tensor_scalar(
                            out=yt, in0=py[:, g, :], scalar1=coef, scalar2=None,
                            op0=mybir.AluOpType.mult,
                        )
                        nc.scalar.dma_start(
                            out=outp[:, g * HGW:(g + 1) * HGW], in_=yt
                        )
                mm2_first[0] = False

            # ---- software-pipelined emission ----
            # A=mm1, B=transpose, C=mm2_partial; each B trails its A by one
            # unit (hides the scalar/vector swiglu latency), each C trails
            # its B. A2 leads so the PE start is gated on w1 block 2.
            mm1(2)
            mm1(0)
            transp(2)
            mm1(1)
            transp(0)
            mm1(3)
            transp(1)
            mm2_partial(0)
            mm1(4)
            transp(3)
            mm2_partial(1)
            mm1(5)
            transp(4)
            mm2_partial(2)
            mm1(6)
            transp(5)
            mm2_partial(3)
            mm1(7)
            transp(6)
            mm2_partial(4)
            transp(7)
            mm2_partial(5)
            mm2_partial(6)
            mm2_partial(7, last=True)

    nc.finalize()
    return nc


_NC = None


def _get_nc():
    global _NC
    if _NC is None:
        _NC = build_bass()
    return _NC


def prep_inputs(hidden_states, routing, w1, w2):
    """Host-side shard + relayout + bf16 cast. Returns in_maps for 8 cores."""
    import ml_dtypes

    bf16 = ml_dtypes.bfloat16
    hs = np.asarray(hidden_states, dtype=np.float32).astype(bf16)
    rt = np.ascontiguousarray(np.asarray(routing, dtype=np.float32))
    w1b = np.asarray(w1, dtype=np.float32).astype(bf16)
    w2b = np.asarray(w2, dtype=np.float32).astype(bf16)

    # hst[p, k, t] = hs[t, k*128+p]
    hst = np.ascontiguousarray(hs.T.reshape(KH, 128, TOKENS).transpose(1, 0, 2))
    # w1s[e, b, p, u, k, o_l] = w1[e, u*4096 + b*512 + o_l, k*128 + p]
    w1p = np.ascontiguousarray(
        w1b.reshape(NEXP, 2, NB, 512, KH, 128).transpose(0, 2, 5, 1, 4, 3)
    )
    # w2s[e, q, p, kl, g, h_l] = w2[e, g*512 + h_l, (q*8 + kl)*128 + p]
    w2p = np.ascontiguousarray(
        w2b.reshape(NEXP, HG, HGW, KQ, KI // KQ, 128).transpose(0, 3, 5, 4, 1, 2)
    )

    in_maps = []
    for c in range(NCORES):
        in_maps.append({
            "hst": hst,
            "w1s": w1p[c],
            "w2s": w2p[c],
            "routing": rt,
            "rlogit": np.ascontiguousarray(rt[:, c:c + 1]),
        })
    return in_maps


def kernel(hidden_states, routing, w1, w2):
    nc = _get_nc()
    in_maps = prep_inputs(hidden_states, routing, w1, w2)
    res = run_bass_kernel_spmd(nc, in_maps, list(range(NCORES)))
    out = np.zeros((TOKENS, HIDDEN), dtype=np.float32)
    for c in range(NCORES):
        out += res.results[c]["outp"]
    return out
